# revision 1
# baseline (speedup 1.0000x reference)
"""CapsuleConv2d (3x3, stride 1, pad 1) with dynamic routing — Trainium2 Bass kernel.

Problem (hardcoded): x (4, 32, 56, 56) f32, weight (4, 4, 9, 8, 16) f32
  -> out (4, 64, 56, 56) f32.

Sharding: 8 cores = 4 batch x 2 pixel-halves of a zero-padded 58x58 grid.
Each core computes all (P_out, P_in) capsule groups for its half of the
padded pixel grid (7 super-tiles of 2x128 flat padded pixels); the host
unpads and stitches. Padding-garbage pixels are computed but discarded.

Per-core pipeline (per super-tile = 2 blocks of 128 pixels):
  PE    : per block, 9 matmuls per conv tap (stationary = shifted x window
          [32, 128], moving = host-built block-diag weight [32, 256]) into a
          shared priors PSUM slot + 9 accumulating matmuls for the tap-sum
  ACT   : copies each block's priors PSUM->SBUF (frees PSUM, enables GPSIMD)
  DVE/ACT/GPSIMD: 3-iteration dynamic routing in free-dim ops over both
          blocks at once (2x free-dim per instruction amortizes op overhead);
          fp16 pairwise-add trees for the weighted sum, f32 logits path
  DMA   : store routed [128 pix, 2, 64 ch] rows; host transposes to NCHW
"""

import sys

sys.path.insert(0, "/opt/trn_rl_repo")

import numpy as np

import concourse.bacc as bacc
import concourse.mybir as mybir
from concourse.bass_utils import run_bass_kernel_spmd
from concourse.hw_specs import get_activation_tables
from concourse.tile import TileContext

# All ACT funcs used here (Square, Ln, Exp) live in act table 6
# ("natural_log_exp_and_others"), but the table-load pass resolves each func
# to its first-containing table, thrashing between tables 0 and 5 (~1.3us per
# reload, ~60 reloads). Offer the pass only table 6 so it emits one load, and
# pin the emitted id to table 6's real index.
_ACT_TABLE_NAME = "natural_log_exp_and_others"


class _PinnedActBacc(bacc.Bacc):
    def insert_act_table_loads(self):
        tabs = get_activation_tables(self.m.arch)
        names = list(tabs.keys())
        idx = names.index(_ACT_TABLE_NAME)
        only = [(_ACT_TABLE_NAME, tabs[_ACT_TABLE_NAME])]
        bacc._bass_rust.insert_act_table_loads(self, only)
        for bb in self.main_func.blocks:
            for inst in bb.instructions:
                if type(inst).__name__ == "InstLoadActFuncSet":
                    if inst.act_func_set_id != idx:
                        inst.act_func_set_id = idx


F32 = mybir.dt.float32
F16 = mybir.dt.float16
AF = mybir.ActivationFunctionType
ALU = mybir.AluOpType
AX = mybir.AxisListType

# geometry
PIN, LIN, POUT, LOUT, KK = 4, 8, 4, 16, 9
CIN = PIN * LIN          # 32
OPD = POUT * PIN * LOUT  # 256 free cols per tap
HP = 58                  # padded grid side
NPIX = HP * HP           # 3364 padded pixels
TILE = 128
NB = 2                   # pixel blocks fused per super-tile
NST = 7                  # super-tiles per core
CORE_PIX = NST * NB * TILE   # 1792
P0_B = NPIX - CORE_PIX   # 1572: second half start
XW_LEN = CORE_PIX + 2 * 59  # 1910: input window incl. tap halo
NCH = POUT * LOUT        # 64 output channels
XIN_LEN = XW_LEN + KK * OPD  # combined input row: x window + weights


def build_program():
    nc = _PinnedActBacc("TRN2", target_bir_lowering=False)
    xin_d = nc.dram_tensor("xin", [CIN, XIN_LEN], F32, kind="ExternalInput")
    out_d = nc.dram_tensor("out", [CORE_PIX, NCH], F32, kind="ExternalOutput")

    with TileContext(nc) as tc:
        with (
            tc.tile_pool(name="const", bufs=1) as const,
            tc.tile_pool(name="pbig", bufs=1) as pbig,
            tc.tile_pool(name="pbig32", bufs=1) as pbig32,
            tc.tile_pool(name="tbig", bufs=1) as tbig,
            tc.tile_pool(name="small", bufs=3) as small,
            tc.tile_pool(name="outp", bufs=2) as outp,
            tc.tile_pool(name="psum_p", bufs=1, space="PSUM") as psum_p,
            tc.tile_pool(name="psum_s", bufs=1, space="PSUM") as psum_s,
        ):
            xin = const.tile([CIN, XIN_LEN], F32)
            # split the load across DMA queues; weights chunk first
            nc.sync.dma_start(out=xin[:, XW_LEN:], in_=xin_d[:, XW_LEN:])
            nchunk = 3
            cs = (XW_LEN + nchunk - 1) // nchunk
            for ci in range(nchunk):
                lo, hi = ci * cs, min((ci + 1) * cs, XW_LEN)
                nc.sync.dma_start(out=xin[:, lo:hi], in_=xin_d[:, lo:hi])
            xw = xin[:, :XW_LEN]
            wm = xin[:, XW_LEN:]
            eps_t = const.tile([TILE, 1], F32, tag="eps")
            nc.vector.memset(eps_t, 1e-30)
            bias_t = {}
            for val in (1.0, 81.0):
                bt = const.tile([TILE, 1], F32, tag=f"bias{int(val)}")
                nc.vector.memset(bt, val)
                bias_t[val] = bt

            NG = NB * 16  # squash groups per partition across blocks

            def squash_sq(v, sfx, split=False):
                """|s|^2 per (block, group): Square (ACT) + reduce_d (DVE).
                v: [TILE, NB*OPD]-shaped AP (any space). split=True runs it
                per block so the first block's result lands earlier."""
                v2 = small.tile([TILE, NB * OPD], F32, tag="v2" + sfx)
                sq = small.tile([TILE, NG], F32, tag="sq" + sfx)
                sqv = sq.rearrange("p (b g) -> p b g", b=NB)
                v2v = v2.rearrange("p (b g d) -> p b g d", b=NB, d=LOUT)
                vv = v.rearrange("p (b gd) -> p b gd", b=NB)
                for b in range(NB) if split else (slice(None),):
                    nc.scalar.activation(
                        out=v2v[:, b] if split else v2, in_=vv[:, b] if split else v,
                        func=AF.Square,
                    )
                    yield
                    nc.vector.tensor_reduce(
                        out=sqv[:, b] if split else sq,
                        in_=v2v[:, b] if split
                        else v2.rearrange("p (g d) -> p g d", d=LOUT),
                        axis=AX.X, op=ALU.add,
                    )
                    yield
                return sq

            def squash_tail(v, sq, denom_bias, sfx, o_engine=None):
                """Given v = c*s (c = sqrt(denom_bias)) and sq = |v|^2,
                returns outputs = squash(s) = v * sqrt(u)/(u + denom_bias).
                All ACT funcs (Square/Ln/Exp) share one HW table."""
                # g = sqrt(u)/(u+denom_bias) = exp(0.5*ln(u+eps) - ln(u+denom))
                la = small.tile([TILE, NG], F32, tag="la" + sfx)
                nc.scalar.activation(out=la, in_=sq, func=AF.Ln, bias=eps_t[:, :])
                lb = small.tile([TILE, NG], F32, tag="lb" + sfx)
                nc.scalar.activation(
                    out=lb, in_=sq, func=AF.Ln, bias=bias_t[denom_bias][:, :]
                )
                yield
                cc = small.tile([TILE, NG], F32, tag="cc" + sfx)
                nc.vector.scalar_tensor_tensor(
                    out=cc, in0=la, scalar=0.5, in1=lb,
                    op0=ALU.mult, op1=ALU.subtract,
                )
                g = small.tile([TILE, NG], F32, tag="g" + sfx)
                nc.scalar.activation(out=g, in_=cc, func=AF.Exp)
                yield
                o = small.tile([TILE, NB * OPD], F32, tag="o" + sfx)
                (o_engine or nc.vector).tensor_mul(
                    o.rearrange("p (g d) -> p g d", d=LOUT),
                    v.rearrange("p (g d) -> p g d", d=LOUT),
                    g.unsqueeze(2).to_broadcast([TILE, NG, LOUT]),
                )
                yield
                return o

            def squash(v, denom_bias, sfx, o_engine=None):
                sq = yield from squash_sq(v, sfx)
                o = yield from squash_tail(v, sq, denom_bias, sfx, o_engine)
                return o

            def logits_contrib(psb, o, sfx):
                """sum_d priors * outputs -> [TILE, NB*144] laid out (b,k,op).
                Multiplies on GPSIMD (one per block) pipelined against DVE
                reduces; f32 throughout (logits are precision-sensitive)."""
                t = tbig.tile([TILE, NB, KK, OPD], F32, tag="tg" + sfx)
                lr = small.tile([TILE, NB * KK * 16], F32, tag="lr" + sfx)
                lrv = lr.rearrange("p (b k g) -> p b k g", b=NB, k=KK)
                ov = o.rearrange("p (b gd) -> p b gd", b=NB)
                KH = 5
                for b in range(NB):
                    for k0, k1 in ((0, KH), (KH, KK)):
                        nc.gpsimd.tensor_mul(
                            t[:, b, k0:k1],
                            psb[:, b, k0:k1],
                            ov[:, b].unsqueeze(1)
                            .to_broadcast([TILE, k1 - k0, OPD]),
                        )
                        yield
                        nc.vector.tensor_reduce(
                            out=lrv[:, b, k0:k1],
                            in_=t[:, b, k0:k1].rearrange(
                                "p k (g d) -> p k g d", d=LOUT
                            ),
                            axis=AX.X, op=ALU.add,
                        )
                        yield
                return lr

            def softmax_k(lg, sfx):
                """softmax over k of [TILE, NB*144] in (b, k, op) layout."""
                e = small.tile([TILE, NB * KK * 16], F32, tag="e" + sfx)
                nc.scalar.activation(out=e, in_=lg, func=AF.Exp)
                yield
                z = small.tile([TILE, NG], F32, tag="z" + sfx)
                nc.vector.tensor_reduce(
                    out=z,
                    in_=e.rearrange("p (b k g) -> p b g k", b=NB, k=KK),
                    axis=AX.X, op=ALU.add,
                )
                zr = small.tile([TILE, NG], F32, tag="zr" + sfx)
                nc.vector.reciprocal(out=zr, in_=z)
                yield
                # probs stored fp16 with k innermost: [b, g, k] so the
                # weighted multiply runs in the DVE 2x packed mode
                pr = small.tile([TILE, NB, 16, KK], F16, tag="pr" + sfx)
                nc.vector.tensor_mul(
                    pr.rearrange("p b g k -> p b k g"),
                    e.rearrange("p (b k g) -> p b k g", b=NB, k=KK),
                    zr.rearrange("p (b g) -> p b g", b=NB)
                    .unsqueeze(2)
                    .to_broadcast([TILE, NB, KK, 16]),
                )
                yield
                return pr

            def weighted_s(psb, pr, sfx):
                """sum_k probs * priors -> [TILE, NB*256]. Both multiply
                operands are fp16 with unit-stride innermost k, so the DVE
                runs its 2x packed mode; k-sum via fp16 pairwise tree over
                the contiguous innermost axis."""
                t = tbig.tile([TILE, NB, 16, LOUT, KK], F16, tag="tt" + sfx)
                tm = t.rearrange("p b g d k -> p (b g) d k")
                nc.vector.tensor_mul(
                    tm,
                    psb.rearrange("p b g d k -> p (b g) d k"),
                    pr.rearrange("p b g k -> p (b g) k")
                    .unsqueeze(2)
                    .to_broadcast([TILE, NB * 16, LOUT, KK]),
                )
                yield
                u1 = tbig.tile([TILE, NB, 16, LOUT, 4], F16, tag="u1" + sfx)
                u1m = u1.rearrange("p b g d k -> p (b g) d k")
                nc.vector.tensor_add(u1m, tm[:, :, :, 0:4], tm[:, :, :, 4:8])
                yield
                u2 = tbig.tile([TILE, NB, 16, LOUT, 2], F16, tag="u2" + sfx)
                u2m = u2.rearrange("p b g d k -> p (b g) d k")
                nc.vector.tensor_add(u2m, u1m[:, :, :, 0:2], u1m[:, :, :, 2:4])
                yield
                u3 = tbig.tile([TILE, NB, 16, LOUT], F16, tag="u3" + sfx)
                u3m = u3.rearrange("p b g d -> p (b g) d")
                nc.vector.tensor_add(u3m, u2m[:, :, :, 0], u2m[:, :, :, 1])
                yield
                v = small.tile([TILE, NB * OPD], F32, tag="v" + sfx)
                nc.vector.tensor_add(
                    v.rearrange("p (bg d) -> p bg d", d=LOUT),
                    u3m,
                    tm[:, :, :, 8],
                )
                yield
                return v

            def tile_body(st, sfx):
                # ---- tap-sums s0 for both blocks (iter-0 needs only these) --
                s0 = psum_s.tile([TILE, NB, OPD], F32, tag="s0" + sfx)
                for b in range(NB):
                    t = st * NB + b
                    for k in range(KK):
                        dj, dk = divmod(k, 3)
                        off = 59 + t * TILE + (dj - 1) * HP + (dk - 1)
                        nc.tensor.matmul(
                            s0[:, b],
                            xw[:, off:off + TILE],
                            wm[:, k * OPD:(k + 1) * OPD],
                            start=(k == 0), stop=(k == KK - 1),
                        )
                        yield
                sq0 = yield from squash_sq(
                    s0.rearrange("p b gd -> p (b gd)"), sfx, split=True
                )
                # ---- per-tap priors, block by block through the shared PSUM
                # slot; ACT copies each block out to SBUF fp32 ----
                # two priors copies: f32 [b,k,g,d] for the precision-
                # sensitive logits path, fp16 k-innermost [b,g,d,k] for the
                # 2x-mode weighted multiplies / contiguous k-trees
                psb32 = pbig32.tile([TILE, NB, KK, OPD], F32, tag="q" + sfx)
                psb16 = pbig.tile(
                    [TILE, NB, 16, LOUT, KK], F16, tag="psb" + sfx
                )
                for b in range(NB):
                    t = st * NB + b
                    pp = psum_p.tile([TILE, KK, OPD], F32, tag="pp")
                    for k in range(KK):
                        dj, dk = divmod(k, 3)
                        off = 59 + t * TILE + (dj - 1) * HP + (dk - 1)
                        nc.tensor.matmul(
                            pp[:, k, :],
                            xw[:, off:off + TILE],
                            wm[:, k * OPD:(k + 1) * OPD],
                            start=True, stop=True,
                        )
                        yield
                    nc.scalar.copy(out=psb32[:, b], in_=pp)
                    yield
                    nc.scalar.copy(
                        out=psb16[:, b].rearrange("p g d k -> p k g d"),
                        in_=psb32[:, b].rearrange(
                            "p k (g d) -> p k g d", d=LOUT
                        ),
                    )
                    yield

                # ---- routing iter 0: probs uniform, s = s0/9; squash folds
                # the 1/9 via denom_bias=81 ----
                o0 = yield from squash_tail(
                    s0.rearrange("p b gd -> p (b gd)"), sq0, 81.0, sfx
                )
                l1 = yield from logits_contrib(psb32, o0, sfx)
                # ---- iter 1 ----
                pr1 = yield from softmax_k(l1, sfx)
                v1 = yield from weighted_s(psb16, pr1, sfx)
                o1 = yield from squash(v1, 1.0, sfx, o_engine=nc.gpsimd)
                l2c = yield from logits_contrib(psb32, o1, sfx)
                l2 = small.tile([TILE, NB * KK * 16], F32, tag="l2" + sfx)
                nc.vector.tensor_add(l2, l1, l2c)
                yield
                # ---- iter 2 ----
                pr2 = yield from softmax_k(l2, sfx)
                v2 = yield from weighted_s(psb16, pr2, sfx)
                o2 = yield from squash(v2, 1.0, sfx)
                # ---- sum over input planes p, store [pix, ch] rows ----
                r = outp.tile([TILE, NB, NCH], F32, tag="rr" + sfx)
                nc.vector.tensor_reduce(
                    out=r,
                    in_=o2.rearrange(
                        "p (b o q d) -> p b o d q", b=NB, o=POUT, q=PIN
                    ),
                    axis=AX.X, op=ALU.add,
                )
                yield
                nc.sync.dma_start(
                    out=out_d[st * NB * TILE:(st + 1) * NB * TILE, :]
                    .rearrange("(b p) c -> p b c", b=NB),
                    in_=r,
                )

            # Interleave instruction emission with a sliding window of two
            # super-tiles so each engine's in-order queue alternates between
            # independent dependency chains.
            gens = []
            nxt = 0
            while gens or nxt < NST:
                while len(gens) < 2 and nxt < NST:
                    gens.append(tile_body(nxt, "AB"[nxt % 2]))
                    nxt += 1
                for gn in list(gens):
                    try:
                        next(gn)
                    except StopIteration:
                        gens.remove(gn)
    nc.compile()
    return nc


_PROG = None


def _get_prog():
    global _PROG
    if _PROG is None:
        _PROG = build_program()
    return _PROG


def _make_inputs(x, weight):
    # block-diagonal moving weights: [c=(p,l), (k, o, p, d)]
    wmov = np.zeros((CIN, KK, POUT, PIN, LOUT), np.float32)
    for p in range(PIN):
        # rows p*LIN..p*LIN+LIN-1 hold weight[o, p, k, l, d]
        wmov[p * LIN:(p + 1) * LIN, :, :, p, :] = np.transpose(
            weight[:, p], (2, 1, 0, 3)
        )  # (l, k, o, d) from (o, k, l, d)
    wmov = wmov.reshape(CIN, KK * OPD)

    xp = np.pad(x, ((0, 0), (0, 0), (1, 1), (1, 1))).reshape(4, CIN, NPIX)
    xpm = np.pad(xp, ((0, 0), (0, 0), (64, 64)))
    in_maps = []
    for c in range(8):
        n, half = divmod(c, 2)
        p0 = 0 if half == 0 else P0_B
        lo = 64 + p0 - 59
        xin = np.concatenate([xpm[n][:, lo:lo + XW_LEN], wmov], axis=1)
        in_maps.append({"xin": np.ascontiguousarray(xin)})
    return in_maps


def _assemble(results):
    out = np.empty((4, NCH, 56, 56), np.float32)
    for n in range(4):
        full = np.empty((NCH, NPIX), np.float32)
        full[:, :CORE_PIX] = results[2 * n]["out"].T
        full[:, CORE_PIX:] = results[2 * n + 1]["out"].T[:, CORE_PIX - P0_B:]
        out[n] = full.reshape(NCH, HP, HP)[:, 1:57, 1:57]
    return out


def kernel(x, weight):
    x = np.asarray(x, np.float32)
    weight = np.asarray(weight, np.float32)
    in_maps = _make_inputs(x, weight)
    last_err = None
    for _ in range(3):  # retry transient NRT/device errors
        try:
            res = run_bass_kernel_spmd(
                _get_prog(), in_maps, core_ids=list(range(8))
            )
            return _assemble(res.results)
        except Exception as e:  # noqa: BLE001
            last_err = e
    raise last_err


if __name__ == "__main__":
    rng = np.random.default_rng(0)
    x = rng.standard_normal((4, 32, 56, 56), dtype=np.float32)
    w = rng.standard_normal((4, 4, 9, 8, 16), dtype=np.float32)
    y = kernel(x, w)
    print("out", y.shape, y.dtype, float(np.abs(y).mean()))



# revision 7
# speedup vs baseline: 1.5133x; 1.5133x over previous
"""CapsuleConv2d (3x3, stride 1, pad 1) with dynamic routing — Trainium2 Bass kernel.

Problem (hardcoded): x (4, 32, 56, 56) f32, weight (4, 4, 9, 8, 16) f32
  -> out (4, 64, 56, 56) f32.

Sharding: 8 cores = 4 batch x 2 pixel-halves of a zero-padded 58x58 grid.
Each core computes all capsule groups for its half of the padded pixel
grid (7 super-tiles of 2x128 flat padded pixels); the host unpads and
stitches. Padding-garbage pixels are computed but discarded.

v2 layout notes ("d-major"): weight columns are (k, d, o, p) so PE
produces priors as [pix, k, d, g] (g=(o,p) innermost). Two fp16 SBUF
copies of the priors serve the two routing contractions with every big
DVE op in the packed-fp16 2x mode:
  psb16d [b, k, d, g] — logits path: mult by o[b,d,g] (bcast over k,
          g innermost packed), then a d-halving add-tree (slices keep
          g innermost).
  psb16k [b, d, g, k] — weighted-sum path: mult by probs[b,g,k] (bcast
          over d, k innermost packed), then a k add-tree.
Matmuls run as float32r (1 cycle/row at >=256 free vs 4 for fp32).
Engine split: PE matmuls; ACT PSUM->SBUF copies + Square/Ln/Exp; Pool
(GPSIMD) the strided psb16k copies; DVE everything else.
"""

import sys

sys.path.insert(0, "/opt/trn_rl_repo")

import numpy as np

import concourse.bacc as bacc
import concourse.mybir as mybir
from concourse.bass_utils import run_bass_kernel_spmd
from concourse.hw_specs import get_activation_tables
from concourse.tile import TileContext

# All ACT funcs used here (Square, Ln, Exp) live in act table 6
# ("natural_log_exp_and_others"), but the table-load pass resolves each func
# to its first-containing table, thrashing between tables. Offer the pass
# only table 6 so it emits one load, and pin the emitted id.
_ACT_TABLE_NAME = "natural_log_exp_and_others"


class _PinnedActBacc(bacc.Bacc):
    def insert_act_table_loads(self):
        tabs = get_activation_tables(self.m.arch)
        names = list(tabs.keys())
        idx = names.index(_ACT_TABLE_NAME)
        only = [(_ACT_TABLE_NAME, tabs[_ACT_TABLE_NAME])]
        bacc._bass_rust.insert_act_table_loads(self, only)
        for bb in self.main_func.blocks:
            for inst in bb.instructions:
                if type(inst).__name__ == "InstLoadActFuncSet":
                    if inst.act_func_set_id != idx:
                        inst.act_func_set_id = idx


F32 = mybir.dt.float32
F32R = mybir.dt.float32r
F16 = mybir.dt.float16
AF = mybir.ActivationFunctionType
ALU = mybir.AluOpType
AX = mybir.AxisListType

# geometry
PIN, LIN, POUT, LOUT, KK = 4, 8, 4, 16, 9
CIN = PIN * LIN          # 32
G = POUT * PIN           # 16 capsule groups (o, p)
OPD = LOUT * G           # 256 free cols per tap, laid out (d, g)
HP = 58                  # padded grid side
NPIX = HP * HP           # 3364 padded pixels
TILE = 128
NB = 2                   # pixel blocks fused per super-tile
NST = 7                  # super-tiles per core
CORE_PIX = NST * NB * TILE   # 1792
P0_B = NPIX - CORE_PIX   # 1572: second half start
XW_LEN = CORE_PIX + 2 * 59  # 1910: input window incl. tap halo
NCH = POUT * LOUT        # 64 output channels
XIN_LEN = XW_LEN + KK * OPD  # combined input row: x window + weights


def build_program():
    nc = _PinnedActBacc("TRN2", target_bir_lowering=False)
    xin_d = nc.dram_tensor("xin", [CIN, XIN_LEN], F32R, kind="ExternalInput")
    out_d = nc.dram_tensor("out", [CORE_PIX, NCH], F32, kind="ExternalOutput")

    with TileContext(nc) as tc:
        with (
            tc.tile_pool(name="const", bufs=1) as const,
            tc.tile_pool(name="pbd", bufs=1) as pbd,
            tc.tile_pool(name="pbk", bufs=1) as pbk,
            tc.tile_pool(name="tbig", bufs=1) as tbig,
            tc.tile_pool(name="small", bufs=1) as small,
            tc.tile_pool(name="outp", bufs=2) as outp,
            tc.tile_pool(name="psum_p", bufs=1, space="PSUM") as psum_p,
            tc.tile_pool(name="psum_s", bufs=1, space="PSUM") as psum_s,
        ):
            xin = const.tile([CIN, XIN_LEN], F32R)
            # split the load across DMA queues; weights chunk first
            nc.sync.dma_start(out=xin[:, XW_LEN:], in_=xin_d[:, XW_LEN:])
            nchunk = 3
            cs = (XW_LEN + nchunk - 1) // nchunk
            for ci in range(nchunk):
                lo, hi = ci * cs, min((ci + 1) * cs, XW_LEN)
                nc.sync.dma_start(out=xin[:, lo:hi], in_=xin_d[:, lo:hi])
            xw = xin[:, :XW_LEN]
            wm = xin[:, XW_LEN:]
            eps_t = const.tile([TILE, 1], F32, tag="eps")
            nc.vector.memset(eps_t, 1e-30)
            bias_t = {}
            for val in (1.0, 81.0):
                bt = const.tile([TILE, 1], F32, tag=f"bias{int(val)}")
                nc.vector.memset(bt, val)
                bias_t[val] = bt

            def mm(out, k, t, start, stop):
                dj, dk = divmod(k, 3)
                off = 59 + t * TILE + (dj - 1) * HP + (dk - 1)
                nc.tensor.matmul(
                    out,
                    xw[:, off:off + TILE],
                    wm[:, k * OPD:(k + 1) * OPD],
                    start=start, stop=stop,
                )

            def squash_scale(s, denom, sfx, zr=None):
                """s [TILE, NB, LOUT, G] fp16 (unnormalized if zr given) ->
                c fp16 [TILE, NB, G]: the squash scale; zr (f32 [.,NB,G])
                is the softmax normalizer folded in (c = g(u)*zr with
                u = |s*zr|^2)."""
                q = small.tile([TILE, NB, LOUT, G], F16, tag="q" + sfx)
                nc.scalar.activation(out=q, in_=s, func=AF.Square)
                yield
                # d-tree: halve the (middle) d axis; g stays innermost
                d1 = small.tile([TILE, NB, 8, G], F16, tag="d1" + sfx)
                nc.vector.tensor_add(d1, q[:, :, 0:8], q[:, :, 8:16])
                yield
                d2 = small.tile([TILE, NB, 4, G], F16, tag="d2" + sfx)
                nc.vector.tensor_add(d2, d1[:, :, 0:4], d1[:, :, 4:8])
                yield
                d3 = small.tile([TILE, NB, 2, G], F16, tag="d3" + sfx)
                nc.vector.tensor_add(d3, d2[:, :, 0:2], d2[:, :, 2:4])
                yield
                u = small.tile([TILE, NB, G], F32, tag="u" + sfx)
                nc.vector.tensor_add(u, d3[:, :, 0], d3[:, :, 1])
                yield
                if zr is not None:
                    zr2 = small.tile([TILE, NB, G], F32, tag="zr2" + sfx)
                    nc.vector.tensor_mul(zr2, zr, zr)
                    nc.vector.tensor_mul(u, u, zr2)
                    yield
                la = small.tile([TILE, NB, G], F32, tag="la" + sfx)
                nc.scalar.activation(out=la, in_=u, func=AF.Ln,
                                     bias=eps_t[:, :])
                lb = small.tile([TILE, NB, G], F32, tag="lb" + sfx)
                nc.scalar.activation(out=lb, in_=u, func=AF.Ln,
                                     bias=bias_t[denom][:, :])
                yield
                cc = small.tile([TILE, NB, G], F32, tag="cc" + sfx)
                nc.vector.scalar_tensor_tensor(
                    out=cc, in0=la, scalar=0.5, in1=lb,
                    op0=ALU.mult, op1=ALU.subtract,
                )
                yield
                gg = small.tile([TILE, NB, G], F32, tag="gg" + sfx)
                nc.scalar.activation(out=gg, in_=cc, func=AF.Exp)
                yield
                c = small.tile([TILE, NB, G], F16, tag="c" + sfx)
                if zr is not None:
                    nc.vector.tensor_mul(c, gg, zr)
                else:
                    nc.vector.tensor_copy(c, gg)
                yield
                return c

            def logits_delta(psb16d, o, sfx):
                """sum_d psb16d[b,k,d,g] * o[b,d,g] -> f32 [TILE,NB,KK,G]."""
                t = tbig.tile([TILE, NB, KK, LOUT, G], F16, tag="t" + sfx[-1])
                nc.vector.tensor_mul(
                    t, psb16d,
                    o.unsqueeze(2).to_broadcast([TILE, NB, KK, LOUT, G]),
                )
                yield
                t1 = tbig.tile([TILE, NB, KK, 8, G], F16, tag="t1" + sfx[-1])
                nc.vector.tensor_add(t1, t[:, :, :, 0:8], t[:, :, :, 8:16])
                yield
                t2 = small.tile([TILE, NB, KK, 4, G], F16, tag="t2" + sfx)
                nc.vector.tensor_add(t2, t1[:, :, :, 0:4], t1[:, :, :, 4:8])
                yield
                t3 = small.tile([TILE, NB, KK, 2, G], F16, tag="t3" + sfx)
                nc.vector.tensor_add(t3, t2[:, :, :, 0:2], t2[:, :, :, 2:4])
                yield
                ld = small.tile([TILE, NB, KK, G], F32, tag="ld" + sfx)
                nc.vector.tensor_add(ld, t3[:, :, :, 0], t3[:, :, :, 1])
                yield
                return ld

            def softmax_zr(l, sfx):
                """softmax pieces over k of l [TILE,NB,KK,G] (f32):
                e16 fp16 [b,g,k] = exp(l) (transposed by ACT), zr f32
                [b,g] = 1/sum_k e. probs = e*zr folded downstream."""
                e = small.tile([TILE, NB, G, KK], F32, tag="e" + sfx)
                nc.scalar.activation(
                    out=e.rearrange("p b g k -> p b k g"), in_=l, func=AF.Exp
                )
                yield
                z1 = small.tile([TILE, NB, G, 4], F32, tag="z1" + sfx)
                nc.vector.tensor_add(z1, e[:, :, :, 0:4], e[:, :, :, 4:8])
                yield
                z2 = small.tile([TILE, NB, G, 2], F32, tag="z2" + sfx)
                nc.vector.tensor_add(z2, z1[:, :, :, 0:2], z1[:, :, :, 2:4])
                yield
                z = small.tile([TILE, NB, G], F32, tag="z" + sfx)
                nc.vector.tensor_add(z, z2[:, :, :, 0], z2[:, :, :, 1])
                yield
                nc.vector.tensor_add(z, z, e[:, :, :, 8])
                yield
                zr = small.tile([TILE, NB, G], F32, tag="zrec" + sfx)
                nc.vector.reciprocal(out=zr, in_=z)
                yield
                e16 = small.tile([TILE, NB, G, KK], F16, tag="e16" + sfx)
                nc.vector.tensor_mul(
                    e16, e,
                    zr.unsqueeze(3).to_broadcast([TILE, NB, G, KK]),
                )
                yield
                return e16, zr

            def weighted_sum(psb16k, e16, sfx):
                """sum_k psb16k[b,d,g,k] * e16[b,g,k] -> fp16 [b,d,g]
                (unnormalized by zr; folded into the squash scale)."""
                tm = tbig.tile([TILE, NB, LOUT, G, KK], F16, tag="tm" + sfx[-1])
                nc.vector.tensor_mul(
                    tm, psb16k,
                    e16.unsqueeze(2).to_broadcast([TILE, NB, LOUT, G, KK]),
                )
                yield
                k1 = tbig.tile([TILE, NB, LOUT, G, 4], F16, tag="k1" + sfx[-1])
                nc.vector.tensor_add(k1, tm[:, :, :, :, 0:4],
                                     tm[:, :, :, :, 4:8])
                yield
                k2 = small.tile([TILE, NB, LOUT, G, 2], F16, tag="k2" + sfx)
                nc.vector.tensor_add(k2, k1[:, :, :, :, 0:2],
                                     k1[:, :, :, :, 2:4])
                yield
                k3 = small.tile([TILE, NB, LOUT, G], F16, tag="k3" + sfx)
                nc.vector.tensor_add(k3, k2[:, :, :, :, 0], k2[:, :, :, :, 1])
                yield
                s = small.tile([TILE, NB, LOUT, G], F16, tag="s" + sfx)
                nc.vector.tensor_add(s, k3, tm[:, :, :, :, 8])
                yield
                return s

            def tile_body(st, sfx):
                # ---- PE: tap-sum s0 (cols (d,g)) + per-tap priors ----
                s0 = psum_s.tile([TILE, NB, OPD], F32, tag="s0" + sfx)
                for b in range(NB):
                    t = st * NB + b
                    for k in range(KK):
                        mm(s0[:, b], k, t, k == 0, k == KK - 1)
                        yield
                s016 = small.tile([TILE, NB, LOUT, G], F16, tag="s016" + sfx)
                nc.scalar.copy(
                    out=s016,
                    in_=s0.rearrange("p b (d g) -> p b d g", g=G),
                )
                yield
                psb16d = pbd.tile([TILE, NB, KK, LOUT, G], F16, tag="pd" + sfx)
                psb16k = pbk.tile([TILE, NB, LOUT, G, KK], F16, tag="pk" + sfx)
                for b in range(NB):
                    t = st * NB + b
                    pp = psum_p.tile([TILE, KK, OPD], F32, tag="pp")
                    for k in range(KK):
                        mm(pp[:, k, :], k, t, True, True)
                        yield
                    nc.scalar.copy(
                        out=psb16d[:, b],
                        in_=pp.rearrange("p k (d g) -> p k d g", g=G),
                    )
                    yield
                    nc.gpsimd.tensor_copy(
                        psb16k[:, b],
                        psb16d[:, b].rearrange("p k d g -> p d g k"),
                    )
                    yield

                # ---- iter 0: probs uniform; squash(s0/9) via denom 81 ----
                c0 = yield from squash_scale(s016, 81.0, "a" + sfx)
                o0 = small.tile([TILE, NB, LOUT, G], F16, tag="o0" + sfx)
                nc.vector.tensor_mul(
                    o0, s016,
                    c0.unsqueeze(2).to_broadcast([TILE, NB, LOUT, G]),
                )
                yield
                l1 = yield from logits_delta(psb16d, o0, "a" + sfx)
                # ---- iter 1 ----
                e1, zr1 = yield from softmax_zr(l1, "a" + sfx)
                s1 = yield from weighted_sum(psb16k, e1, "a" + sfx)
                c1 = yield from squash_scale(s1, 1.0, "b" + sfx)
                o1 = small.tile([TILE, NB, LOUT, G], F16, tag="o1" + sfx)
                nc.vector.tensor_mul(
                    o1, s1,
                    c1.unsqueeze(2).to_broadcast([TILE, NB, LOUT, G]),
                )
                yield
                ld2 = yield from logits_delta(psb16d, o1, "b" + sfx)
                l2 = small.tile([TILE, NB, KK, G], F32, tag="l2" + sfx)
                nc.vector.tensor_add(l2, l1, ld2)
                yield
                # ---- iter 2 ----
                e2, zr2 = yield from softmax_zr(l2, "b" + sfx)
                s2 = yield from weighted_sum(psb16k, e2, "b" + sfx)
                c2 = yield from squash_scale(s2, 1.0, "c" + sfx)
                o2 = small.tile([TILE, NB, LOUT, G], F16, tag="o2" + sfx)
                nc.vector.tensor_mul(
                    o2, s2,
                    c2.unsqueeze(2).to_broadcast([TILE, NB, LOUT, G]),
                )
                yield
                # ---- sum over input planes p (innermost of g=(o,p)) ----
                o2v = o2.rearrange("p b d (o q) -> p b d o q", q=PIN)
                po = small.tile([TILE, NB, LOUT, POUT, 2], F16,
                                tag="po" + sfx)
                nc.vector.tensor_add(po, o2v[:, :, :, :, 0:2],
                                     o2v[:, :, :, :, 2:4])
                yield
                r = outp.tile([TILE, NB, NCH], F32, tag="rr" + sfx)
                nc.vector.tensor_add(
                    r.rearrange("p b (o d) -> p b d o", d=LOUT),
                    po[:, :, :, :, 0], po[:, :, :, :, 1],
                )
                yield
                nc.sync.dma_start(
                    out=out_d[st * NB * TILE:(st + 1) * NB * TILE, :]
                    .rearrange("(b p) c -> p b c", b=NB),
                    in_=r,
                )

            # Interleave instruction emission with a sliding window of two
            # super-tiles so each engine's in-order queue alternates between
            # independent dependency chains.
            gens = []
            nxt = 0
            while gens or nxt < NST:
                while len(gens) < 2 and nxt < NST:
                    gens.append(tile_body(nxt, "AB"[nxt % 2]))
                    nxt += 1
                for gn in list(gens):
                    try:
                        next(gn)
                    except StopIteration:
                        gens.remove(gn)
    nc.compile()
    return nc


_PROG = None


def _get_prog():
    global _PROG
    if _PROG is None:
        _PROG = build_program()
    return _PROG


def _make_inputs(x, weight):
    # block-diagonal moving weights: [c=(p,l), (k, d, o, p)]
    wmov = np.zeros((CIN, KK, LOUT, POUT, PIN), np.float32)
    for p in range(PIN):
        # weight[:, p]: (o, k, l, d) -> (l, k, d, o) into rows p*LIN..+LIN
        wmov[p * LIN:(p + 1) * LIN, :, :, :, p] = np.transpose(
            weight[:, p], (2, 1, 3, 0)
        )
    wmov = wmov.reshape(CIN, KK * OPD)

    xp = np.pad(x, ((0, 0), (0, 0), (1, 1), (1, 1))).reshape(4, CIN, NPIX)
    xpm = np.pad(xp, ((0, 0), (0, 0), (64, 64)))
    in_maps = []
    for c in range(8):
        n, half = divmod(c, 2)
        p0 = 0 if half == 0 else P0_B
        lo = 64 + p0 - 59
        xin = np.concatenate([xpm[n][:, lo:lo + XW_LEN], wmov], axis=1)
        in_maps.append({"xin": np.ascontiguousarray(xin)})
    return in_maps


def _assemble(results):
    out = np.empty((4, NCH, 56, 56), np.float32)
    for n in range(4):
        full = np.empty((NCH, NPIX), np.float32)
        full[:, :CORE_PIX] = results[2 * n]["out"].T
        full[:, CORE_PIX:] = results[2 * n + 1]["out"].T[:, CORE_PIX - P0_B:]
        out[n] = full.reshape(NCH, HP, HP)[:, 1:57, 1:57]
    return out


def kernel(x, weight):
    x = np.asarray(x, np.float32)
    weight = np.asarray(weight, np.float32)
    in_maps = _make_inputs(x, weight)
    last_err = None
    for _ in range(3):  # retry transient NRT/device errors
        try:
            res = run_bass_kernel_spmd(
                _get_prog(), in_maps, core_ids=list(range(8))
            )
            return _assemble(res.results)
        except Exception as e:  # noqa: BLE001
            last_err = e
    raise last_err


if __name__ == "__main__":
    rng = np.random.default_rng(0)
    x = rng.standard_normal((4, 32, 56, 56), dtype=np.float32)
    w = rng.standard_normal((4, 4, 9, 8, 16), dtype=np.float32)
    y = kernel(x, w)
    print("out", y.shape, y.dtype, float(np.abs(y).mean()))


# revision 8
# speedup vs baseline: 1.5653x; 1.0343x over previous
"""CapsuleConv2d (3x3, stride 1, pad 1) with dynamic routing — Trainium2 Bass kernel.

Problem (hardcoded): x (4, 32, 56, 56) f32, weight (4, 4, 9, 8, 16) f32
  -> out (4, 64, 56, 56) f32.

Sharding: 8 cores = 4 batch x 2 pixel-halves of a zero-padded 58x58 grid.
Each core computes all capsule groups for its half of the padded pixel
grid (7 super-tiles of 2x128 flat padded pixels); the host unpads and
stitches. Padding-garbage pixels are computed but discarded.

v3 layout notes ("d-major, single priors copy"): weight columns are
(k, d, o, p) so PE (float32r, 1 cycle/row) writes priors as
[pix, k, d, g] with g=(o,p) innermost. ONE fp16 SBUF copy psb16d
[b, k, d, g] serves both routing contractions in the DVE packed-fp16
2x mode, because every per-pixel routing vector broadcast (outputs
o[b,d,g] over k; probs e16[b,k,g] over d) happens on a NON-innermost
axis while g stays innermost/packed:
  logits path: t = psb16d * s[b,d,g] (bcast k) -> d-halving tree;
          the squash scale c[b,g] is folded in AFTER the reduce
          (l = c * sum_d P*s), shortening the critical chain.
  weighted path: tm = psb16d * e16[b,k,g] (bcast d) -> k tree over
          the outer k axis -> s[b,d,g].
Engine split: PE f32r matmuls; ACT PSUM->SBUF fp16 copies and
Square/Ln/Exp (one pinned act table); DVE everything else.
"""

import sys

sys.path.insert(0, "/opt/trn_rl_repo")

import numpy as np

import concourse.bacc as bacc
import concourse.mybir as mybir
from concourse.bass_utils import run_bass_kernel_spmd
from concourse.hw_specs import get_activation_tables
from concourse.tile import TileContext

# All ACT funcs used here (Square, Ln, Exp) live in act table 6
# ("natural_log_exp_and_others"); offer the table-load pass only that
# table so it emits a single load, and pin the emitted id.
_ACT_TABLE_NAME = "natural_log_exp_and_others"


class _PinnedActBacc(bacc.Bacc):
    def insert_act_table_loads(self):
        tabs = get_activation_tables(self.m.arch)
        names = list(tabs.keys())
        idx = names.index(_ACT_TABLE_NAME)
        only = [(_ACT_TABLE_NAME, tabs[_ACT_TABLE_NAME])]
        bacc._bass_rust.insert_act_table_loads(self, only)
        for bb in self.main_func.blocks:
            for inst in bb.instructions:
                if type(inst).__name__ == "InstLoadActFuncSet":
                    if inst.act_func_set_id != idx:
                        inst.act_func_set_id = idx


F32 = mybir.dt.float32
F32R = mybir.dt.float32r
F16 = mybir.dt.float16
AF = mybir.ActivationFunctionType
ALU = mybir.AluOpType
AX = mybir.AxisListType

# geometry
PIN, LIN, POUT, LOUT, KK = 4, 8, 4, 16, 9
CIN = PIN * LIN          # 32
G = POUT * PIN           # 16 capsule groups (o, p)
OPD = LOUT * G           # 256 free cols per tap, laid out (d, g)
HP = 58                  # padded grid side
NPIX = HP * HP           # 3364 padded pixels
TILE = 128
NB = 2                   # pixel blocks fused per super-tile
NST = 7                  # super-tiles per core
CORE_PIX = NST * NB * TILE   # 1792
P0_B = NPIX - CORE_PIX   # 1572: second half start
XW_LEN = CORE_PIX + 2 * 59  # 1910: input window incl. tap halo
NCH = POUT * LOUT        # 64 output channels
XIN_LEN = XW_LEN + KK * OPD  # combined input row: x window + weights


def build_program():
    nc = _PinnedActBacc("TRN2", target_bir_lowering=False)
    xin_d = nc.dram_tensor("xin", [CIN, XIN_LEN], F32R, kind="ExternalInput")
    out_d = nc.dram_tensor("out", [CORE_PIX, NCH], F32, kind="ExternalOutput")

    with TileContext(nc) as tc:
        with (
            tc.tile_pool(name="const", bufs=1) as const,
            tc.tile_pool(name="pbd", bufs=1) as pbd,
            tc.tile_pool(name="tbig", bufs=1) as tbig,
            tc.tile_pool(name="small", bufs=1) as small,
            tc.tile_pool(name="outp", bufs=2) as outp,
            tc.tile_pool(name="psum_p", bufs=1, space="PSUM") as psum_p,
            tc.tile_pool(name="psum_s", bufs=1, space="PSUM") as psum_s,
        ):
            xin = const.tile([CIN, XIN_LEN], F32R)
            # split the load across DMA queues; weights chunk first
            nc.sync.dma_start(out=xin[:, XW_LEN:], in_=xin_d[:, XW_LEN:])
            nchunk = 3
            cs = (XW_LEN + nchunk - 1) // nchunk
            for ci in range(nchunk):
                lo, hi = ci * cs, min((ci + 1) * cs, XW_LEN)
                nc.sync.dma_start(out=xin[:, lo:hi], in_=xin_d[:, lo:hi])
            xw = xin[:, :XW_LEN]
            wm = xin[:, XW_LEN:]
            eps_t = const.tile([TILE, 1], F32, tag="eps")
            nc.vector.memset(eps_t, 1e-30)
            bias_t = {}
            for val in (1.0, 81.0):
                bt = const.tile([TILE, 1], F32, tag=f"bias{int(val)}")
                nc.vector.memset(bt, val)
                bias_t[val] = bt

            def mm(out, k, t, start, stop):
                dj, dk = divmod(k, 3)
                off = 59 + t * TILE + (dj - 1) * HP + (dk - 1)
                nc.tensor.matmul(
                    out,
                    xw[:, off:off + TILE],
                    wm[:, k * OPD:(k + 1) * OPD],
                    start=start, stop=stop,
                )

            def squash_scale(s, denom, sfx):
                """s [TILE, NB, LOUT, G] fp16 -> c fp16 [TILE, NB, G]:
                c = sqrt(u)/(u+denom) with u = |s|^2 (sum over d)."""
                q = small.tile([TILE, NB, LOUT, G], F16, tag="q" + sfx)
                nc.scalar.activation(out=q, in_=s, func=AF.Square)
                yield
                # d-tree: halve the (middle) d axis; g stays innermost
                d1 = small.tile([TILE, NB, 8, G], F16, tag="d1" + sfx)
                nc.vector.tensor_add(d1, q[:, :, 0:8], q[:, :, 8:16])
                yield
                d2 = small.tile([TILE, NB, 4, G], F16, tag="d2" + sfx)
                nc.vector.tensor_add(d2, d1[:, :, 0:4], d1[:, :, 4:8])
                yield
                d3 = small.tile([TILE, NB, 2, G], F16, tag="d3" + sfx)
                nc.vector.tensor_add(d3, d2[:, :, 0:2], d2[:, :, 2:4])
                yield
                u = small.tile([TILE, NB, G], F32, tag="u" + sfx)
                nc.vector.tensor_add(u, d3[:, :, 0], d3[:, :, 1])
                yield
                la = small.tile([TILE, NB, G], F32, tag="la" + sfx)
                nc.scalar.activation(out=la, in_=u, func=AF.Ln,
                                     bias=eps_t[:, :])
                lb = small.tile([TILE, NB, G], F32, tag="lb" + sfx)
                nc.scalar.activation(out=lb, in_=u, func=AF.Ln,
                                     bias=bias_t[denom][:, :])
                yield
                cc = small.tile([TILE, NB, G], F32, tag="cc" + sfx)
                nc.vector.scalar_tensor_tensor(
                    out=cc, in0=la, scalar=0.5, in1=lb,
                    op0=ALU.mult, op1=ALU.subtract,
                )
                yield
                c = small.tile([TILE, NB, G], F16, tag="c" + sfx)
                nc.scalar.activation(out=c, in_=cc, func=AF.Exp)
                yield
                return c

            def logits_raw(psb16d, s, sfx):
                """sum_d psb16d[b,k,d,g] * s[b,d,g] -> f32 [TILE,NB,KK,G]
                (the squash scale is applied by the caller afterwards)."""
                t = tbig.tile([TILE, NB, KK, LOUT, G], F16, tag="t" + sfx[-1])
                nc.vector.tensor_mul(
                    t, psb16d,
                    s.unsqueeze(2).to_broadcast([TILE, NB, KK, LOUT, G]),
                )
                yield
                t1 = tbig.tile([TILE, NB, KK, 8, G], F16,
                               tag="t1" + sfx[-1])
                nc.vector.tensor_add(t1, t[:, :, :, 0:8], t[:, :, :, 8:16])
                yield
                t2 = small.tile([TILE, NB, KK, 4, G], F16, tag="t2" + sfx)
                nc.vector.tensor_add(t2, t1[:, :, :, 0:4], t1[:, :, :, 4:8])
                yield
                t3 = small.tile([TILE, NB, KK, 2, G], F16, tag="t3" + sfx)
                nc.vector.tensor_add(t3, t2[:, :, :, 0:2], t2[:, :, :, 2:4])
                yield
                lr = small.tile([TILE, NB, KK, G], F32, tag="lr" + sfx)
                nc.vector.tensor_add(lr, t3[:, :, :, 0], t3[:, :, :, 1])
                yield
                return lr

            def softmax_e16(l, sfx):
                """probs over k of l [TILE,NB,KK,G] f32 -> fp16 [b,k,g]."""
                e = small.tile([TILE, NB, KK, G], F32, tag="e" + sfx)
                nc.scalar.activation(out=e, in_=l, func=AF.Exp)
                yield
                z1 = small.tile([TILE, NB, 4, G], F32, tag="z1" + sfx)
                nc.vector.tensor_add(z1, e[:, :, 0:4], e[:, :, 4:8])
                yield
                z2 = small.tile([TILE, NB, 2, G], F32, tag="z2" + sfx)
                nc.vector.tensor_add(z2, z1[:, :, 0:2], z1[:, :, 2:4])
                yield
                z = small.tile([TILE, NB, G], F32, tag="z" + sfx)
                nc.vector.tensor_add(z, z2[:, :, 0], z2[:, :, 1])
                yield
                nc.vector.tensor_add(z, z, e[:, :, 8])
                yield
                zr = small.tile([TILE, NB, G], F32, tag="zrec" + sfx)
                nc.vector.reciprocal(out=zr, in_=z)
                yield
                e16 = small.tile([TILE, NB, KK, G], F16, tag="e16" + sfx)
                nc.vector.tensor_mul(
                    e16, e,
                    zr.unsqueeze(2).to_broadcast([TILE, NB, KK, G]),
                )
                yield
                return e16

            def weighted_sum(psb16d, e16, sfx):
                """sum_k psb16d[b,k,d,g] * e16[b,k,g] -> fp16 [b,d,g]."""
                tm = tbig.tile([TILE, NB, KK, LOUT, G], F16,
                               tag="tm" + sfx[-1])
                nc.vector.tensor_mul(
                    tm, psb16d,
                    e16.unsqueeze(3).to_broadcast([TILE, NB, KK, LOUT, G]),
                )
                yield
                k1 = tbig.tile([TILE, NB, 4, LOUT, G], F16,
                               tag="k1" + sfx[-1])
                nc.vector.tensor_add(k1, tm[:, :, 0:4], tm[:, :, 4:8])
                yield
                k2 = small.tile([TILE, NB, 2, LOUT, G], F16, tag="k2" + sfx)
                nc.vector.tensor_add(k2, k1[:, :, 0:2], k1[:, :, 2:4])
                yield
                k3 = small.tile([TILE, NB, LOUT, G], F16, tag="k3" + sfx)
                nc.vector.tensor_add(k3, k2[:, :, 0], k2[:, :, 1])
                yield
                s = small.tile([TILE, NB, LOUT, G], F16, tag="s" + sfx)
                nc.vector.tensor_add(s, k3, tm[:, :, 8])
                yield
                return s

            def tile_body(st, sfx):
                # ---- PE: tap-sum s0 (cols (d,g)) + per-tap priors ----
                s0 = psum_s.tile([TILE, NB, OPD], F32, tag="s0" + sfx)
                for b in range(NB):
                    t = st * NB + b
                    for k in range(KK):
                        mm(s0[:, b], k, t, k == 0, k == KK - 1)
                        yield
                s016 = small.tile([TILE, NB, LOUT, G], F16, tag="s016" + sfx)
                nc.scalar.copy(
                    out=s016,
                    in_=s0.rearrange("p b (d g) -> p b d g", g=G),
                )
                yield
                psb16d = pbd.tile([TILE, NB, KK, LOUT, G], F16, tag="pd" + sfx)
                for b in range(NB):
                    t = st * NB + b
                    pp = psum_p.tile([TILE, KK, OPD], F32, tag="pp")
                    for k in range(KK):
                        mm(pp[:, k, :], k, t, True, True)
                        yield
                    nc.scalar.copy(
                        out=psb16d[:, b],
                        in_=pp.rearrange("p k (d g) -> p k d g", g=G),
                    )
                    yield

                # ---- iter 0: probs uniform; squash(s0/9) via denom 81.
                # The big mult uses raw s0 and the scale c0 is folded in
                # after the d-reduce: l1 = c0 * sum_d P*s0.
                lr1 = yield from logits_raw(psb16d, s016, "a" + sfx)
                c0 = yield from squash_scale(s016, 81.0, "a" + sfx)
                l1 = small.tile([TILE, NB, KK, G], F32, tag="l1" + sfx)
                nc.vector.tensor_mul(
                    l1, lr1,
                    c0.unsqueeze(2).to_broadcast([TILE, NB, KK, G]),
                )
                yield
                # ---- iter 1 ----
                e1 = yield from softmax_e16(l1, "a" + sfx)
                s1 = yield from weighted_sum(psb16d, e1, "a" + sfx)
                lr2 = yield from logits_raw(psb16d, s1, "b" + sfx)
                c1 = yield from squash_scale(s1, 1.0, "b" + sfx)
                l2 = small.tile([TILE, NB, KK, G], F32, tag="l2" + sfx)
                nc.vector.tensor_mul(
                    l2, lr2,
                    c1.unsqueeze(2).to_broadcast([TILE, NB, KK, G]),
                )
                yield
                nc.vector.tensor_add(l2, l2, l1)
                yield
                # ---- iter 2 ----
                e2 = yield from softmax_e16(l2, "b" + sfx)
                s2 = yield from weighted_sum(psb16d, e2, "b" + sfx)
                c2 = yield from squash_scale(s2, 1.0, "c" + sfx)
                o2 = small.tile([TILE, NB, LOUT, G], F16, tag="o2" + sfx)
                nc.vector.tensor_mul(
                    o2, s2,
                    c2.unsqueeze(2).to_broadcast([TILE, NB, LOUT, G]),
                )
                yield
                # ---- sum over input planes p (innermost of g=(o,p)) ----
                o2v = o2.rearrange("p b d (o q) -> p b d o q", q=PIN)
                po = small.tile([TILE, NB, LOUT, POUT, 2], F16,
                                tag="po" + sfx)
                nc.vector.tensor_add(po, o2v[:, :, :, :, 0:2],
                                     o2v[:, :, :, :, 2:4])
                yield
                r = outp.tile([TILE, NB, NCH], F32, tag="rr" + sfx)
                nc.vector.tensor_add(
                    r.rearrange("p b (o d) -> p b d o", d=LOUT),
                    po[:, :, :, :, 0], po[:, :, :, :, 1],
                )
                yield
                nc.sync.dma_start(
                    out=out_d[st * NB * TILE:(st + 1) * NB * TILE, :]
                    .rearrange("(b p) c -> p b c", b=NB),
                    in_=r,
                )

            # Interleave instruction emission with a sliding window of
            # super-tiles so each engine's in-order queue alternates between
            # independent dependency chains.
            DEPTH = 2
            gens = []
            nxt = 0
            while gens or nxt < NST:
                while len(gens) < DEPTH and nxt < NST:
                    gens.append(tile_body(nxt, "AB"[nxt % DEPTH]))
                    nxt += 1
                for gn in list(gens):
                    try:
                        next(gn)
                    except StopIteration:
                        gens.remove(gn)
    nc.compile()
    return nc


_PROG = None


def _get_prog():
    global _PROG
    if _PROG is None:
        _PROG = build_program()
    return _PROG


def _make_inputs(x, weight):
    # block-diagonal moving weights: [c=(p,l), (k, d, o, p)]
    wmov = np.zeros((CIN, KK, LOUT, POUT, PIN), np.float32)
    for p in range(PIN):
        # weight[:, p]: (o, k, l, d) -> (l, k, d, o) into rows p*LIN..+LIN
        wmov[p * LIN:(p + 1) * LIN, :, :, :, p] = np.transpose(
            weight[:, p], (2, 1, 3, 0)
        )
    wmov = wmov.reshape(CIN, KK * OPD)

    xp = np.pad(x, ((0, 0), (0, 0), (1, 1), (1, 1))).reshape(4, CIN, NPIX)
    xpm = np.pad(xp, ((0, 0), (0, 0), (64, 64)))
    in_maps = []
    for c in range(8):
        n, half = divmod(c, 2)
        p0 = 0 if half == 0 else P0_B
        lo = 64 + p0 - 59
        xin = np.concatenate([xpm[n][:, lo:lo + XW_LEN], wmov], axis=1)
        in_maps.append({"xin": np.ascontiguousarray(xin)})
    return in_maps


def _assemble(results):
    out = np.empty((4, NCH, 56, 56), np.float32)
    for n in range(4):
        full = np.empty((NCH, NPIX), np.float32)
        full[:, :CORE_PIX] = results[2 * n]["out"].T
        full[:, CORE_PIX:] = results[2 * n + 1]["out"].T[:, CORE_PIX - P0_B:]
        out[n] = full.reshape(NCH, HP, HP)[:, 1:57, 1:57]
    return out


def kernel(x, weight):
    x = np.asarray(x, np.float32)
    weight = np.asarray(weight, np.float32)
    in_maps = _make_inputs(x, weight)
    last_err = None
    for _ in range(3):  # retry transient NRT/device errors
        try:
            res = run_bass_kernel_spmd(
                _get_prog(), in_maps, core_ids=list(range(8))
            )
            return _assemble(res.results)
        except Exception as e:  # noqa: BLE001
            last_err = e
    raise last_err


if __name__ == "__main__":
    rng = np.random.default_rng(0)
    x = rng.standard_normal((4, 32, 56, 56), dtype=np.float32)
    w = rng.standard_normal((4, 4, 9, 8, 16), dtype=np.float32)
    y = kernel(x, w)
    print("out", y.shape, y.dtype, float(np.abs(y).mean()))


# revision 10
# speedup vs baseline: 1.6720x; 1.0682x over previous
"""CapsuleConv2d (3x3, stride 1, pad 1) with dynamic routing — Trainium2 Bass kernel.

Problem (hardcoded): x (4, 32, 56, 56) f32, weight (4, 4, 9, 8, 16) f32
  -> out (4, 64, 56, 56) f32.

Sharding: 8 cores = 4 batch x 2 pixel-halves of a zero-padded 58x58 grid.
Each core computes all capsule groups for its half of the padded pixel
grid (7 super-tiles of 2x128 flat padded pixels); the host unpads and
stitches. Padding-garbage pixels are computed but discarded.

v3 layout notes ("d-major, single priors copy"): weight columns are
(k, d, o, p) so PE (float32r, 1 cycle/row) writes priors as
[pix, k, d, g] with g=(o,p) innermost. ONE fp16 SBUF copy psb16d
[b, k, d, g] serves both routing contractions in the DVE packed-fp16
2x mode, because every per-pixel routing vector broadcast (outputs
o[b,d,g] over k; probs e16[b,k,g] over d) happens on a NON-innermost
axis while g stays innermost/packed:
  logits path: t = psb16d * s[b,d,g] (bcast k) -> d-halving tree;
          the squash scale c[b,g] is folded in AFTER the reduce
          (l = c * sum_d P*s), shortening the critical chain.
  weighted path: tm = psb16d * e16[b,k,g] (bcast d) -> k tree over
          the outer k axis -> s[b,d,g].
Engine split: PE f32r matmuls; ACT PSUM->SBUF fp16 copies and
Square/Ln/Exp (one pinned act table); DVE everything else.
"""

import sys

sys.path.insert(0, "/opt/trn_rl_repo")

import numpy as np

import concourse.bacc as bacc
import concourse.mybir as mybir
from concourse.bass_utils import run_bass_kernel_spmd
from concourse.hw_specs import get_activation_tables
from concourse.tile import TileContext

# All ACT funcs used here (Square, Ln, Exp) live in act table 6
# ("natural_log_exp_and_others"); offer the table-load pass only that
# table so it emits a single load, and pin the emitted id.
_ACT_TABLE_NAME = "natural_log_exp_and_others"


class _PinnedActBacc(bacc.Bacc):
    def insert_act_table_loads(self):
        tabs = get_activation_tables(self.m.arch)
        names = list(tabs.keys())
        idx = names.index(_ACT_TABLE_NAME)
        only = [(_ACT_TABLE_NAME, tabs[_ACT_TABLE_NAME])]
        bacc._bass_rust.insert_act_table_loads(self, only)
        for bb in self.main_func.blocks:
            for inst in bb.instructions:
                if type(inst).__name__ == "InstLoadActFuncSet":
                    if inst.act_func_set_id != idx:
                        inst.act_func_set_id = idx


F32 = mybir.dt.float32
F32R = mybir.dt.float32r
F16 = mybir.dt.float16
AF = mybir.ActivationFunctionType
ALU = mybir.AluOpType
AX = mybir.AxisListType

# geometry
PIN, LIN, POUT, LOUT, KK = 4, 8, 4, 16, 9
CIN = PIN * LIN          # 32
G = POUT * PIN           # 16 capsule groups (o, p)
OPD = LOUT * G           # 256 free cols per tap, laid out (d, g)
HP = 58                  # padded grid side
NPIX = HP * HP           # 3364 padded pixels
TILE = 128
NB = 2                   # pixel blocks fused per super-tile
NST = 7                  # super-tiles per core
CORE_PIX = NST * NB * TILE   # 1792
P0_B = NPIX - CORE_PIX   # 1572: second half start
XW_LEN = CORE_PIX + 2 * 59  # 1910: input window incl. tap halo
NCH = POUT * LOUT        # 64 output channels
XIN_LEN = XW_LEN + KK * OPD  # combined input row: x window + weights


def build_program():
    nc = _PinnedActBacc("TRN2", target_bir_lowering=False)
    xin_d = nc.dram_tensor("xin", [CIN, XIN_LEN], F32R, kind="ExternalInput")
    out_d = nc.dram_tensor("out", [CORE_PIX, NCH], F32, kind="ExternalOutput")

    with TileContext(nc) as tc:
        with (
            tc.tile_pool(name="const", bufs=1) as const,
            tc.tile_pool(name="pbd", bufs=1) as pbd,
            tc.tile_pool(name="tbig", bufs=1) as tbig,
            tc.tile_pool(name="small", bufs=1) as small,
            tc.tile_pool(name="outp", bufs=2) as outp,
            tc.tile_pool(name="psum_p", bufs=1, space="PSUM") as psum_p,
            tc.tile_pool(name="psum_s", bufs=1, space="PSUM") as psum_s,
        ):
            xin = const.tile([CIN, XIN_LEN], F32R)
            # split the load across DMA queues; first-needed chunks first
            wsp = XW_LEN + 5 * OPD
            nc.sync.dma_start(out=xin[:, XW_LEN:wsp], in_=xin_d[:, XW_LEN:wsp])
            nc.sync.dma_start(out=xin[:, 0:448], in_=xin_d[:, 0:448])
            nc.sync.dma_start(out=xin[:, wsp:], in_=xin_d[:, wsp:])
            nchunk = 3
            cs = (XW_LEN - 448 + nchunk - 1) // nchunk
            for ci in range(nchunk):
                lo, hi = 448 + ci * cs, min(448 + (ci + 1) * cs, XW_LEN)
                nc.sync.dma_start(out=xin[:, lo:hi], in_=xin_d[:, lo:hi])
            xw = xin[:, :XW_LEN]
            wm = xin[:, XW_LEN:]
            eps_t = const.tile([TILE, 1], F32, tag="eps")
            nc.vector.memset(eps_t, 1e-30)
            bias_t = {}
            for val in (1.0, 81.0):
                bt = const.tile([TILE, 1], F32, tag=f"bias{int(val)}")
                nc.vector.memset(bt, val)
                bias_t[val] = bt

            def mm(out, k, t, start, stop):
                dj, dk = divmod(k, 3)
                off = 59 + t * TILE + (dj - 1) * HP + (dk - 1)
                nc.tensor.matmul(
                    out,
                    xw[:, off:off + TILE],
                    wm[:, k * OPD:(k + 1) * OPD],
                    start=start, stop=stop,
                )

            def squash_scale(s, denom, sfx):
                """s [TILE, NB, LOUT, G] fp16 -> c fp16 [TILE, NB, G]:
                c = sqrt(u)/(u+denom) with u = |s|^2 (sum over d)."""
                q = small.tile([TILE, NB, LOUT, G], F16, tag="q" + sfx)
                nc.scalar.activation(out=q, in_=s, func=AF.Square)
                yield
                # d-tree: halve the (middle) d axis; g stays innermost
                d1 = small.tile([TILE, NB, 8, G], F16, tag="d1" + sfx)
                nc.gpsimd.tensor_add(d1, q[:, :, 0:8], q[:, :, 8:16])
                yield
                d2 = small.tile([TILE, NB, 4, G], F16, tag="d2" + sfx)
                nc.gpsimd.tensor_add(d2, d1[:, :, 0:4], d1[:, :, 4:8])
                yield
                d3 = small.tile([TILE, NB, 2, G], F16, tag="d3" + sfx)
                nc.gpsimd.tensor_add(d3, d2[:, :, 0:2], d2[:, :, 2:4])
                yield
                u = small.tile([TILE, NB, G], F32, tag="u" + sfx)
                nc.gpsimd.tensor_add(u, d3[:, :, 0], d3[:, :, 1])
                yield
                la = small.tile([TILE, NB, G], F32, tag="la" + sfx)
                nc.scalar.activation(out=la, in_=u, func=AF.Ln,
                                     bias=eps_t[:, :])
                lb = small.tile([TILE, NB, G], F32, tag="lb" + sfx)
                nc.scalar.activation(out=lb, in_=u, func=AF.Ln,
                                     bias=bias_t[denom][:, :])
                yield
                cc = small.tile([TILE, NB, G], F32, tag="cc" + sfx)
                nc.vector.scalar_tensor_tensor(
                    out=cc, in0=la, scalar=0.5, in1=lb,
                    op0=ALU.mult, op1=ALU.subtract,
                )
                yield
                c = small.tile([TILE, NB, G], F16, tag="c" + sfx)
                nc.scalar.activation(out=c, in_=cc, func=AF.Exp)
                yield
                return c

            def logits_raw(psb16d, s, sfx):
                """sum_d psb16d[b,k,d,g] * s[b,d,g] -> f32 [TILE,NB,KK,G]
                (the squash scale is applied by the caller afterwards)."""
                t = tbig.tile([TILE, NB, KK, LOUT, G], F16, tag="t" + sfx[-1])
                for b in range(NB):
                    nc.vector.tensor_mul(
                        t[:, b], psb16d[:, b],
                        s[:, b].unsqueeze(1)
                        .to_broadcast([TILE, KK, LOUT, G]),
                    )
                    yield
                t1 = tbig.tile([TILE, NB, KK, 8, G], F16,
                               tag="t1" + sfx[-1])
                for b in range(NB):
                    nc.vector.tensor_add(t1[:, b], t[:, b, :, 0:8],
                                         t[:, b, :, 8:16])
                    yield
                t2 = small.tile([TILE, NB, KK, 4, G], F16, tag="t2" + sfx)
                nc.vector.tensor_add(t2, t1[:, :, :, 0:4], t1[:, :, :, 4:8])
                yield
                t3 = small.tile([TILE, NB, KK, 2, G], F16, tag="t3" + sfx)
                nc.vector.tensor_add(t3, t2[:, :, :, 0:2], t2[:, :, :, 2:4])
                yield
                lr = small.tile([TILE, NB, KK, G], F32, tag="lr" + sfx)
                nc.vector.tensor_add(lr, t3[:, :, :, 0], t3[:, :, :, 1])
                yield
                return lr

            def softmax_e16(l, sfx):
                """probs over k of l [TILE,NB,KK,G] f32 -> fp16 [b,k,g]."""
                e = small.tile([TILE, NB, KK, G], F32, tag="e" + sfx)
                nc.scalar.activation(out=e, in_=l, func=AF.Exp)
                yield
                z1 = small.tile([TILE, NB, 4, G], F32, tag="z1" + sfx)
                nc.gpsimd.tensor_add(z1, e[:, :, 0:4], e[:, :, 4:8])
                yield
                z2 = small.tile([TILE, NB, 2, G], F32, tag="z2" + sfx)
                nc.gpsimd.tensor_add(z2, z1[:, :, 0:2], z1[:, :, 2:4])
                yield
                z = small.tile([TILE, NB, G], F32, tag="z" + sfx)
                nc.gpsimd.tensor_add(z, z2[:, :, 0], z2[:, :, 1])
                yield
                nc.gpsimd.tensor_add(z, z, e[:, :, 8])
                yield
                zr = small.tile([TILE, NB, G], F32, tag="zrec" + sfx)
                nc.vector.reciprocal(out=zr, in_=z)
                yield
                e16 = small.tile([TILE, NB, KK, G], F16, tag="e16" + sfx)
                nc.vector.tensor_mul(
                    e16, e,
                    zr.unsqueeze(2).to_broadcast([TILE, NB, KK, G]),
                )
                yield
                return e16

            def weighted_sum(psb16d, e16, sfx):
                """sum_k psb16d[b,k,d,g] * e16[b,k,g] -> fp16 [b,d,g]."""
                tm = tbig.tile([TILE, NB, KK, LOUT, G], F16,
                               tag="tm" + sfx[-1])
                for b in range(NB):
                    nc.vector.tensor_mul(
                        tm[:, b], psb16d[:, b],
                        e16[:, b].unsqueeze(2)
                        .to_broadcast([TILE, KK, LOUT, G]),
                    )
                    yield
                k1 = tbig.tile([TILE, NB, 4, LOUT, G], F16,
                               tag="k1" + sfx[-1])
                for b in range(NB):
                    nc.vector.tensor_add(k1[:, b], tm[:, b, 0:4],
                                         tm[:, b, 4:8])
                    yield
                k2 = small.tile([TILE, NB, 2, LOUT, G], F16, tag="k2" + sfx)
                nc.vector.tensor_add(k2, k1[:, :, 0:2], k1[:, :, 2:4])
                yield
                k3 = small.tile([TILE, NB, LOUT, G], F16, tag="k3" + sfx)
                nc.vector.tensor_add(k3, k2[:, :, 0], k2[:, :, 1])
                yield
                s = small.tile([TILE, NB, LOUT, G], F16, tag="s" + sfx)
                nc.vector.tensor_add(s, k3, tm[:, :, 8])
                yield
                return s

            def tile_body(st, sfx):
                # ---- PE: tap-sum s0 (cols (d,g)) + per-tap priors ----
                s0 = psum_s.tile([TILE, NB, OPD], F32, tag="s0" + sfx)
                s016 = small.tile([TILE, NB, LOUT, G], F16, tag="s016" + sfx)
                psb16d = pbd.tile([TILE, NB, KK, LOUT, G], F16, tag="pd" + sfx)
                for b in range(NB):
                    t = st * NB + b
                    for k in range(KK):
                        mm(s0[:, b], k, t, k == 0, k == KK - 1)
                        yield
                    nc.scalar.copy(
                        out=s016[:, b],
                        in_=s0[:, b].rearrange("p (d g) -> p d g", g=G),
                    )
                    yield
                    for k0, k1 in ((0, 5), (5, KK)):
                        pp = psum_p.tile([TILE, k1 - k0, OPD], F32,
                                         tag=f"pp{k0}")
                        for k in range(k0, k1):
                            mm(pp[:, k - k0, :], k, t, True, True)
                            yield
                        nc.scalar.copy(
                            out=psb16d[:, b, k0:k1],
                            in_=pp.rearrange("p k (d g) -> p k d g", g=G),
                        )
                        yield

                # ---- iter 0: probs uniform; squash(s0/9) via denom 81.
                # The big mult uses raw s0 and the scale c0 is folded in
                # after the d-reduce: l1 = c0 * sum_d P*s0.
                lr1 = yield from logits_raw(psb16d, s016, "a" + sfx)
                c0 = yield from squash_scale(s016, 81.0, "a" + sfx)
                l1 = small.tile([TILE, NB, KK, G], F32, tag="l1" + sfx)
                nc.vector.tensor_mul(
                    l1, lr1,
                    c0.unsqueeze(2).to_broadcast([TILE, NB, KK, G]),
                )
                yield
                # ---- iter 1 ----
                e1 = yield from softmax_e16(l1, "a" + sfx)
                s1 = yield from weighted_sum(psb16d, e1, "a" + sfx)
                lr2 = yield from logits_raw(psb16d, s1, "b" + sfx)
                c1 = yield from squash_scale(s1, 1.0, "b" + sfx)
                l2 = small.tile([TILE, NB, KK, G], F32, tag="l2" + sfx)
                nc.vector.tensor_mul(
                    l2, lr2,
                    c1.unsqueeze(2).to_broadcast([TILE, NB, KK, G]),
                )
                yield
                nc.vector.tensor_add(l2, l2, l1)
                yield
                # ---- iter 2 ----
                e2 = yield from softmax_e16(l2, "b" + sfx)
                s2 = yield from weighted_sum(psb16d, e2, "b" + sfx)
                c2 = yield from squash_scale(s2, 1.0, "c" + sfx)
                o2 = small.tile([TILE, NB, LOUT, G], F16, tag="o2" + sfx)
                nc.vector.tensor_mul(
                    o2, s2,
                    c2.unsqueeze(2).to_broadcast([TILE, NB, LOUT, G]),
                )
                yield
                # ---- sum over input planes p (innermost of g=(o,p)) ----
                o2v = o2.rearrange("p b d (o q) -> p b d o q", q=PIN)
                po = small.tile([TILE, NB, LOUT, POUT, 2], F16,
                                tag="po" + sfx)
                nc.gpsimd.tensor_add(po, o2v[:, :, :, :, 0:2],
                                     o2v[:, :, :, :, 2:4])
                yield
                r = outp.tile([TILE, NB, NCH], F32, tag="rr" + sfx)
                nc.gpsimd.tensor_add(
                    r.rearrange("p b (o d) -> p b d o", d=LOUT),
                    po[:, :, :, :, 0], po[:, :, :, :, 1],
                )
                yield
                nc.sync.dma_start(
                    out=out_d[st * NB * TILE:(st + 1) * NB * TILE, :]
                    .rearrange("(b p) c -> p b c", b=NB),
                    in_=r,
                )

            # Interleave instruction emission with a sliding window of
            # super-tiles so each engine's in-order queue alternates between
            # independent dependency chains.
            DEPTH = 2
            gens = []
            nxt = 0
            while gens or nxt < NST:
                while len(gens) < DEPTH and nxt < NST:
                    gens.append(tile_body(nxt, "AB"[nxt % DEPTH]))
                    nxt += 1
                for gn in list(gens):
                    try:
                        next(gn)
                    except StopIteration:
                        gens.remove(gn)
    nc.compile()
    return nc


_PROG = None


def _get_prog():
    global _PROG
    if _PROG is None:
        _PROG = build_program()
    return _PROG


def _make_inputs(x, weight):
    # block-diagonal moving weights: [c=(p,l), (k, d, o, p)]
    wmov = np.zeros((CIN, KK, LOUT, POUT, PIN), np.float32)
    for p in range(PIN):
        # weight[:, p]: (o, k, l, d) -> (l, k, d, o) into rows p*LIN..+LIN
        wmov[p * LIN:(p + 1) * LIN, :, :, :, p] = np.transpose(
            weight[:, p], (2, 1, 3, 0)
        )
    wmov = wmov.reshape(CIN, KK * OPD)

    xp = np.pad(x, ((0, 0), (0, 0), (1, 1), (1, 1))).reshape(4, CIN, NPIX)
    xpm = np.pad(xp, ((0, 0), (0, 0), (64, 64)))
    in_maps = []
    for c in range(8):
        n, half = divmod(c, 2)
        p0 = 0 if half == 0 else P0_B
        lo = 64 + p0 - 59
        xin = np.concatenate([xpm[n][:, lo:lo + XW_LEN], wmov], axis=1)
        in_maps.append({"xin": np.ascontiguousarray(xin)})
    return in_maps


def _assemble(results):
    out = np.empty((4, NCH, 56, 56), np.float32)
    for n in range(4):
        full = np.empty((NCH, NPIX), np.float32)
        full[:, :CORE_PIX] = results[2 * n]["out"].T
        full[:, CORE_PIX:] = results[2 * n + 1]["out"].T[:, CORE_PIX - P0_B:]
        out[n] = full.reshape(NCH, HP, HP)[:, 1:57, 1:57]
    return out


def kernel(x, weight):
    x = np.asarray(x, np.float32)
    weight = np.asarray(weight, np.float32)
    in_maps = _make_inputs(x, weight)
    last_err = None
    for _ in range(3):  # retry transient NRT/device errors
        try:
            res = run_bass_kernel_spmd(
                _get_prog(), in_maps, core_ids=list(range(8))
            )
            return _assemble(res.results)
        except Exception as e:  # noqa: BLE001
            last_err = e
    raise last_err


if __name__ == "__main__":
    rng = np.random.default_rng(0)
    x = rng.standard_normal((4, 32, 56, 56), dtype=np.float32)
    w = rng.standard_normal((4, 4, 9, 8, 16), dtype=np.float32)
    y = kernel(x, w)
    print("out", y.shape, y.dtype, float(np.abs(y).mean()))


# revision 15
# speedup vs baseline: 1.7123x; 1.0241x over previous
"""CapsuleConv2d (3x3, stride 1, pad 1) with dynamic routing — Trainium2 Bass kernel.

Problem (hardcoded): x (4, 32, 56, 56) f32, weight (4, 4, 9, 8, 16) f32
  -> out (4, 64, 56, 56) f32.

Sharding: 8 cores = 4 batch x 2 pixel-halves of a zero-padded 58x58 grid.
Each core computes all capsule groups for its half of the padded pixel
grid (7 super-tiles of 2x128 flat padded pixels); the host unpads and
stitches. Padding-garbage pixels are computed but discarded.

v3 layout notes ("d-major, single priors copy"): weight columns are
(k, d, o, p) so PE (float32r, 1 cycle/row) writes priors as
[pix, k, d, g] with g=(o,p) innermost. ONE fp16 SBUF copy psb16d
[b, k, d, g] serves both routing contractions in the DVE packed-fp16
2x mode, because every per-pixel routing vector broadcast (outputs
o[b,d,g] over k; probs e16[b,k,g] over d) happens on a NON-innermost
axis while g stays innermost/packed:
  logits path: t = psb16d * s[b,d,g] (bcast k) -> d-halving tree;
          the squash scale c[b,g] is folded in AFTER the reduce
          (l = c * sum_d P*s), shortening the critical chain.
  weighted path: tm = psb16d * e16[b,k,g] (bcast d) -> k tree over
          the outer k axis -> s[b,d,g].
Engine split: PE f32r matmuls; ACT PSUM->SBUF fp16 copies and
Square/Ln/Exp (one pinned act table); DVE everything else.
"""

import sys

sys.path.insert(0, "/opt/trn_rl_repo")

import numpy as np

import concourse.bacc as bacc
import concourse.mybir as mybir
from concourse.bass_utils import run_bass_kernel_spmd
from concourse.hw_specs import get_activation_tables
from concourse.tile import TileContext

# All ACT funcs used here (Square, Ln, Exp) live in act table 6
# ("natural_log_exp_and_others"); offer the table-load pass only that
# table so it emits a single load, and pin the emitted id.
_ACT_TABLE_NAME = "natural_log_exp_and_others"


class _PinnedActBacc(bacc.Bacc):
    def insert_act_table_loads(self):
        tabs = get_activation_tables(self.m.arch)
        names = list(tabs.keys())
        idx = names.index(_ACT_TABLE_NAME)
        only = [(_ACT_TABLE_NAME, tabs[_ACT_TABLE_NAME])]
        bacc._bass_rust.insert_act_table_loads(self, only)
        for bb in self.main_func.blocks:
            for inst in bb.instructions:
                if type(inst).__name__ == "InstLoadActFuncSet":
                    if inst.act_func_set_id != idx:
                        inst.act_func_set_id = idx


F32 = mybir.dt.float32
F32R = mybir.dt.float32r
F16 = mybir.dt.float16
AF = mybir.ActivationFunctionType
ALU = mybir.AluOpType
AX = mybir.AxisListType

# geometry
PIN, LIN, POUT, LOUT, KK = 4, 8, 4, 16, 9
CIN = PIN * LIN          # 32
G = POUT * PIN           # 16 capsule groups (o, p)
OPD = LOUT * G           # 256 free cols per tap, laid out (d, g)
HP = 58                  # padded grid side
NPIX = HP * HP           # 3364 padded pixels
TILE = 128
NB = 2                   # pixel blocks fused per super-tile
NST = 7                  # super-tiles per core
CORE_PIX = NST * NB * TILE   # 1792
P0_B = NPIX - CORE_PIX   # 1572: second half start
XW_LEN = CORE_PIX + 2 * 59  # 1910: input window incl. tap halo
NCH = POUT * LOUT        # 64 output channels
XIN_LEN = XW_LEN + KK * OPD  # combined input row: x window + weights


def build_program():
    nc = _PinnedActBacc("TRN2", target_bir_lowering=False)
    xin_d = nc.dram_tensor("xin", [CIN, XIN_LEN], F32R, kind="ExternalInput")
    out_d = nc.dram_tensor("out", [CORE_PIX, NCH], F32, kind="ExternalOutput")

    with TileContext(nc) as tc:
        with (
            tc.tile_pool(name="const", bufs=1) as const,
            tc.tile_pool(name="pbd", bufs=1) as pbd,
            tc.tile_pool(name="tbig", bufs=1) as tbig,
            tc.tile_pool(name="small", bufs=1) as small,
            tc.tile_pool(name="outp", bufs=2) as outp,
            tc.tile_pool(name="psum_p", bufs=1, space="PSUM") as psum_p,
            tc.tile_pool(name="psum_s", bufs=1, space="PSUM") as psum_s,
        ):
            xin = const.tile([CIN, XIN_LEN], F32R)
            # split the load across DMA queues; first-needed chunks first
            wsp = XW_LEN + 5 * OPD
            nc.sync.dma_start(out=xin[:, XW_LEN:wsp], in_=xin_d[:, XW_LEN:wsp])
            nc.sync.dma_start(out=xin[:, 0:448], in_=xin_d[:, 0:448])
            nc.sync.dma_start(out=xin[:, wsp:], in_=xin_d[:, wsp:])
            nchunk = 3
            cs = (XW_LEN - 448 + nchunk - 1) // nchunk
            for ci in range(nchunk):
                lo, hi = 448 + ci * cs, min(448 + (ci + 1) * cs, XW_LEN)
                nc.sync.dma_start(out=xin[:, lo:hi], in_=xin_d[:, lo:hi])
            xw = xin[:, :XW_LEN]
            wm = xin[:, XW_LEN:]
            eps_t = const.tile([TILE, 1], F32, tag="eps")
            nc.vector.memset(eps_t, 1e-30)
            bias_t = {}
            for val in (1.0, 81.0):
                bt = const.tile([TILE, 1], F32, tag=f"bias{int(val)}")
                nc.vector.memset(bt, val)
                bias_t[val] = bt

            def mm(out, k, t, start, stop):
                dj, dk = divmod(k, 3)
                off = 59 + t * TILE + (dj - 1) * HP + (dk - 1)
                nc.tensor.matmul(
                    out,
                    xw[:, off:off + TILE],
                    wm[:, k * OPD:(k + 1) * OPD],
                    start=start, stop=stop,
                )

            def squash_scale(s, denom, sfx):
                """s [TILE, NB, LOUT, G] fp16 -> c fp16 [TILE, NB, G]:
                c = sqrt(u)/(u+denom) with u = |s|^2 (sum over d)."""
                q = small.tile([TILE, NB, LOUT, G], F16, tag="q" + sfx)
                nc.scalar.activation(out=q, in_=s, func=AF.Square)
                yield
                # d-tree: halve the (middle) d axis; g stays innermost
                d1 = small.tile([TILE, NB, 8, G], F16, tag="d1" + sfx)
                nc.gpsimd.tensor_add(d1, q[:, :, 0:8], q[:, :, 8:16])
                yield
                d2 = small.tile([TILE, NB, 4, G], F16, tag="d2" + sfx)
                nc.gpsimd.tensor_add(d2, d1[:, :, 0:4], d1[:, :, 4:8])
                yield
                d3 = small.tile([TILE, NB, 2, G], F16, tag="d3" + sfx)
                nc.gpsimd.tensor_add(d3, d2[:, :, 0:2], d2[:, :, 2:4])
                yield
                u = small.tile([TILE, NB, G], F32, tag="u" + sfx)
                nc.gpsimd.tensor_add(u, d3[:, :, 0], d3[:, :, 1])
                yield
                la = small.tile([TILE, NB, G], F32, tag="la" + sfx)
                nc.scalar.activation(out=la, in_=u, func=AF.Ln,
                                     bias=eps_t[:, :])
                lb = small.tile([TILE, NB, G], F32, tag="lb" + sfx)
                nc.scalar.activation(out=lb, in_=u, func=AF.Ln,
                                     bias=bias_t[denom][:, :])
                yield
                cc = small.tile([TILE, NB, G], F32, tag="cc" + sfx)
                nc.vector.scalar_tensor_tensor(
                    out=cc, in0=la, scalar=0.5, in1=lb,
                    op0=ALU.mult, op1=ALU.subtract,
                )
                yield
                c = small.tile([TILE, NB, G], F16, tag="c" + sfx)
                nc.scalar.activation(out=c, in_=cc, func=AF.Exp)
                yield
                return c

            def logits_raw(psb16d, s, sfx):
                """sum_d psb16d[b,k,d,g] * s[b,d,g] -> f32 [TILE,NB,KK,G]
                (the squash scale is applied by the caller afterwards)."""
                t = tbig.tile([TILE, NB, KK, LOUT, G], F16, tag="t" + sfx[-1])
                for b in range(NB):
                    nc.vector.tensor_mul(
                        t[:, b], psb16d[:, b],
                        s[:, b].unsqueeze(1)
                        .to_broadcast([TILE, KK, LOUT, G]),
                    )
                    yield
                t1 = tbig.tile([TILE, NB, KK, 8, G], F16,
                               tag="t1" + sfx[-1])
                for b in range(NB):
                    nc.vector.tensor_add(t1[:, b], t[:, b, :, 0:8],
                                         t[:, b, :, 8:16])
                    yield
                t2 = small.tile([TILE, NB, KK, 4, G], F16, tag="t2" + sfx)
                nc.vector.tensor_add(t2, t1[:, :, :, 0:4], t1[:, :, :, 4:8])
                yield
                t3 = small.tile([TILE, NB, KK, 2, G], F16, tag="t3" + sfx)
                nc.vector.tensor_add(t3, t2[:, :, :, 0:2], t2[:, :, :, 2:4])
                yield
                lr = small.tile([TILE, NB, KK, G], F32, tag="lr" + sfx)
                nc.vector.tensor_add(lr, t3[:, :, :, 0], t3[:, :, :, 1])
                yield
                return lr

            def softmax_e16(l, sfx):
                """probs over k of l [TILE,NB,KK,G] f32 -> fp16 [b,k,g]."""
                e = small.tile([TILE, NB, KK, G], F32, tag="e" + sfx)
                nc.scalar.activation(out=e, in_=l, func=AF.Exp)
                yield
                z1 = small.tile([TILE, NB, 4, G], F32, tag="z1" + sfx)
                nc.vector.tensor_add(z1, e[:, :, 0:4], e[:, :, 4:8])
                yield
                z2 = small.tile([TILE, NB, 2, G], F32, tag="z2" + sfx)
                nc.vector.tensor_add(z2, z1[:, :, 0:2], z1[:, :, 2:4])
                yield
                z = small.tile([TILE, NB, G], F32, tag="z" + sfx)
                nc.vector.tensor_add(z, z2[:, :, 0], z2[:, :, 1])
                yield
                nc.vector.tensor_add(z, z, e[:, :, 8])
                yield
                zr = small.tile([TILE, NB, G], F32, tag="zrec" + sfx)
                nc.vector.reciprocal(out=zr, in_=z)
                yield
                e16 = small.tile([TILE, NB, KK, G], F16, tag="e16" + sfx)
                nc.vector.tensor_mul(
                    e16, e,
                    zr.unsqueeze(2).to_broadcast([TILE, NB, KK, G]),
                )
                yield
                return e16

            def weighted_sum(psb16d, e16, sfx):
                """sum_k psb16d[b,k,d,g] * e16[b,k,g] -> fp16 [b,d,g]."""
                tm = tbig.tile([TILE, NB, KK, LOUT, G], F16,
                               tag="tm" + sfx[-1])
                for b in range(NB):
                    nc.vector.tensor_mul(
                        tm[:, b], psb16d[:, b],
                        e16[:, b].unsqueeze(2)
                        .to_broadcast([TILE, KK, LOUT, G]),
                    )
                    yield
                k1 = tbig.tile([TILE, NB, 4, LOUT, G], F16,
                               tag="k1" + sfx[-1])
                for b in range(NB):
                    nc.vector.tensor_add(k1[:, b], tm[:, b, 0:4],
                                         tm[:, b, 4:8])
                    yield
                k2 = small.tile([TILE, NB, 2, LOUT, G], F16, tag="k2" + sfx)
                nc.vector.tensor_add(k2, k1[:, :, 0:2], k1[:, :, 2:4])
                yield
                k3 = small.tile([TILE, NB, LOUT, G], F16, tag="k3" + sfx)
                nc.vector.tensor_add(k3, k2[:, :, 0], k2[:, :, 1])
                yield
                s = small.tile([TILE, NB, LOUT, G], F16, tag="s" + sfx)
                nc.vector.tensor_add(s, k3, tm[:, :, 8])
                yield
                return s

            def tile_body(st, sfx):
                # ---- PE: tap-sum s0 (cols (d,g)) + per-tap priors ----
                s0 = psum_s.tile([TILE, NB, OPD], F32, tag="s0" + sfx)
                s016 = small.tile([TILE, NB, LOUT, G], F16, tag="s016" + sfx)
                psb16d = pbd.tile([TILE, NB, KK, LOUT, G], F16, tag="pd" + sfx)
                for b in range(NB):
                    t = st * NB + b
                    for k in range(KK):
                        mm(s0[:, b], k, t, k == 0, k == KK - 1)
                        yield
                    nc.scalar.copy(
                        out=s016[:, b],
                        in_=s0[:, b].rearrange("p (d g) -> p d g", g=G),
                    )
                    yield
                    for k0, k1 in ((0, 5), (5, KK)):
                        pp = psum_p.tile([TILE, k1 - k0, OPD], F32,
                                         tag=f"pp{k0}")
                        for k in range(k0, k1):
                            mm(pp[:, k - k0, :], k, t, True, True)
                            yield
                        nc.scalar.copy(
                            out=psb16d[:, b, k0:k1],
                            in_=pp.rearrange("p k (d g) -> p k d g", g=G),
                        )
                        yield

                # ---- iter 0: probs uniform; squash(s0/9) via denom 81.
                # The big mult uses raw s0 and the scale c0 is folded in
                # after the d-reduce: l1 = c0 * sum_d P*s0.
                lr1 = yield from logits_raw(psb16d, s016, "a" + sfx)
                c0 = yield from squash_scale(s016, 81.0, "a" + sfx)
                l1 = small.tile([TILE, NB, KK, G], F32, tag="l1" + sfx)
                nc.vector.tensor_mul(
                    l1, lr1,
                    c0.unsqueeze(2).to_broadcast([TILE, NB, KK, G]),
                )
                yield
                # ---- iter 1 ----
                e1 = yield from softmax_e16(l1, "a" + sfx)
                s1 = yield from weighted_sum(psb16d, e1, "a" + sfx)
                lr2 = yield from logits_raw(psb16d, s1, "b" + sfx)
                c1 = yield from squash_scale(s1, 1.0, "b" + sfx)
                l2 = small.tile([TILE, NB, KK, G], F32, tag="l2" + sfx)
                nc.vector.tensor_mul(
                    l2, lr2,
                    c1.unsqueeze(2).to_broadcast([TILE, NB, KK, G]),
                )
                yield
                nc.vector.tensor_add(l2, l2, l1)
                yield
                # ---- iter 2 ----
                e2 = yield from softmax_e16(l2, "b" + sfx)
                s2 = yield from weighted_sum(psb16d, e2, "b" + sfx)
                c2 = yield from squash_scale(s2, 1.0, "c" + sfx)
                o2 = small.tile([TILE, NB, LOUT, G], F16, tag="o2" + sfx)
                nc.vector.tensor_mul(
                    o2, s2,
                    c2.unsqueeze(2).to_broadcast([TILE, NB, LOUT, G]),
                )
                yield
                # ---- sum over input planes p (innermost of g=(o,p)) ----
                o2v = o2.rearrange("p b d (o q) -> p b d o q", q=PIN)
                po = small.tile([TILE, NB, LOUT, POUT, 2], F16,
                                tag="po" + sfx)
                nc.gpsimd.tensor_add(po, o2v[:, :, :, :, 0:2],
                                     o2v[:, :, :, :, 2:4])
                yield
                r = outp.tile([TILE, NB, NCH], F32, tag="rr" + sfx)
                nc.gpsimd.tensor_add(
                    r.rearrange("p b (o d) -> p b d o", d=LOUT),
                    po[:, :, :, :, 0], po[:, :, :, :, 1],
                )
                yield
                nc.sync.dma_start(
                    out=out_d[st * NB * TILE:(st + 1) * NB * TILE, :]
                    .rearrange("(b p) c -> p b c", b=NB),
                    in_=r,
                )

            # Interleave instruction emission with a sliding window of
            # super-tiles so each engine's in-order queue alternates between
            # independent dependency chains.
            DEPTH = 2
            gens = []
            nxt = 0
            while gens or nxt < NST:
                while len(gens) < DEPTH and nxt < NST:
                    gens.append(tile_body(nxt, "AB"[nxt % DEPTH]))
                    nxt += 1
                for gn in list(gens):
                    try:
                        next(gn)
                    except StopIteration:
                        gens.remove(gn)
    nc.compile()
    return nc


_PROG = None


def _get_prog():
    global _PROG
    if _PROG is None:
        _PROG = build_program()
    return _PROG


def _make_inputs(x, weight):
    # block-diagonal moving weights: [c=(p,l), (k, d, o, p)]
    wmov = np.zeros((CIN, KK, LOUT, POUT, PIN), np.float32)
    for p in range(PIN):
        # weight[:, p]: (o, k, l, d) -> (l, k, d, o) into rows p*LIN..+LIN
        wmov[p * LIN:(p + 1) * LIN, :, :, :, p] = np.transpose(
            weight[:, p], (2, 1, 3, 0)
        )
    wmov = wmov.reshape(CIN, KK * OPD)

    xp = np.pad(x, ((0, 0), (0, 0), (1, 1), (1, 1))).reshape(4, CIN, NPIX)
    xpm = np.pad(xp, ((0, 0), (0, 0), (64, 64)))
    in_maps = []
    for c in range(8):
        n, half = divmod(c, 2)
        p0 = 0 if half == 0 else P0_B
        lo = 64 + p0 - 59
        xin = np.concatenate([xpm[n][:, lo:lo + XW_LEN], wmov], axis=1)
        in_maps.append({"xin": np.ascontiguousarray(xin)})
    return in_maps


def _assemble(results):
    out = np.empty((4, NCH, 56, 56), np.float32)
    for n in range(4):
        full = np.empty((NCH, NPIX), np.float32)
        full[:, :CORE_PIX] = results[2 * n]["out"].T
        full[:, CORE_PIX:] = results[2 * n + 1]["out"].T[:, CORE_PIX - P0_B:]
        out[n] = full.reshape(NCH, HP, HP)[:, 1:57, 1:57]
    return out


def kernel(x, weight):
    x = np.asarray(x, np.float32)
    weight = np.asarray(weight, np.float32)
    in_maps = _make_inputs(x, weight)
    last_err = None
    for _ in range(3):  # retry transient NRT/device errors
        try:
            res = run_bass_kernel_spmd(
                _get_prog(), in_maps, core_ids=list(range(8))
            )
            return _assemble(res.results)
        except Exception as e:  # noqa: BLE001
            last_err = e
    raise last_err


if __name__ == "__main__":
    rng = np.random.default_rng(0)
    x = rng.standard_normal((4, 32, 56, 56), dtype=np.float32)
    w = rng.standard_normal((4, 4, 9, 8, 16), dtype=np.float32)
    y = kernel(x, w)
    print("out", y.shape, y.dtype, float(np.abs(y).mean()))
